# revision 1
# baseline (speedup 1.0000x reference)
"""Trainium2 Bass kernel for dual-attention block (CAM + SAM + bottleneck).

Contract: kernel(**inputs) takes FULL unsharded inputs
  x     [8, 64, 64, 64] f32
  w_cam [64, 64], w_q [32, 64], w_k [32, 64], w_v [64, 64], w_bn [64, 128]
and returns the full [8, 64, 64, 64] f32 output.

Sharding: data-parallel over batch across 8 NeuronCores (1 image each);
weights replicated. Per-core math (c=64 channels, n=m=4096 spatial):

  CAM: xcT = x.T @ w_cam.T ; Ec = xcT.T @ xcT (symmetric);
       attn_c = softmax_rows(Ec); out_c = attn_c @ x
  SAM: q4/k4 = (w stacked 4x) @ x  -> q,k replicated on 4 partition groups
       S[m,n] = sum_c k[c,m] q[c,n]  (row-tiled K=32 matmuls, 3 concurrent)
       E = exp(S)  (no max subtraction needed: |S| < ~6)
       acc[c,n] = sum_m W[m,c] E[m,n]  with W = [v.T | ones]
                  -> rows 0..63 unnormalized out_s, row 64 = Z (softmax denom)
  out = x + w_bn[:, :64] @ out_c + (w_bn[:, 64:] @ acc[0:64]) * (1/Z)
        (per-n 1/Z broadcast to 64 partitions via a K=1 PE matmul)
"""

import os
import sys
from contextlib import ExitStack

import numpy as np

if "/opt/trn_rl_repo" not in sys.path:
    sys.path.insert(0, "/opt/trn_rl_repo")

import concourse.bass as bass
import concourse.tile as tile
from concourse import bacc, mybir
from concourse.bass_utils import run_bass_kernel_spmd

F32 = mybir.dt.float32
C = 64          # channels
HW = 4096       # 64*64 spatial
NB = 8          # number of 512-wide n blocks
BLK = 512
MT = 32         # m tiles of 128
GRP = 3         # m-tiles per S/exp group (3 PSUM banks)


def _groups():
    """m-tile groups: [0,3), [3,6), ... sizes 3,3,...,2 covering 32 tiles."""
    out = []
    base = 0
    while base < MT:
        size = min(GRP, MT - base)
        out.append((base, size))
        base += size
    return out


def _build_kernel(ctx: ExitStack, tc: tile.TileContext, io: dict):
    nc = tc.nc
    x_d = io["x"]
    out_d = io["out"]

    consts = ctx.enter_context(tc.tile_pool(name="consts", bufs=1))
    bigs = ctx.enter_context(tc.tile_pool(name="bigs", bufs=1))
    epool = ctx.enter_context(tc.tile_pool(name="epool", bufs=3))
    campool = ctx.enter_context(tc.tile_pool(name="campool", bufs=1))
    sampool = ctx.enter_context(tc.tile_pool(name="sampool", bufs=2))
    outpool = ctx.enter_context(tc.tile_pool(name="outpool", bufs=3))
    spool = ctx.enter_context(
        tc.tile_pool(name="spool", bufs=2, space=bass.MemorySpace.PSUM)
    )
    vpool = ctx.enter_context(
        tc.tile_pool(name="vpool", bufs=1, space=bass.MemorySpace.PSUM)
    )
    ppool = ctx.enter_context(
        tc.tile_pool(name="ppool", bufs=1, space=bass.MemorySpace.PSUM)
    )

    # ---- load constants / inputs ----
    wq4T = consts.tile([C, 128], F32)     # (w_q stacked 4x).T
    wk4T = consts.tile([C, 128], F32)
    wvc = consts.tile([C, 129], F32)      # [v.T | 0 | w_cam.T]
    wbn1T = consts.tile([C, C], F32)
    wbn2T = consts.tile([C, C], F32)
    ident = consts.tile([C, C], F32)
    ones_r = consts.tile([128, C], F32)   # row 64 holds ones[1, 64]
    zbias = consts.tile([128, 1], F32)

    nc.sync.dma_start(wq4T[:], io["wq4T"][:])
    nc.sync.dma_start(wk4T[:], io["wk4T"][:])
    nc.sync.dma_start(wvc[:], io["wvc"][:])
    nc.sync.dma_start(wbn1T[:], io["wbn1T"][:])
    nc.sync.dma_start(wbn2T[:], io["wbn2T"][:])
    nc.sync.dma_start(ident[:], io["ident"][:])
    nc.sync.dma_start(ones_r[C : C + 1, :], io["ones64"][:])
    nc.vector.memset(zbias[:], 0.0)

    x_sb = bigs.tile([C, HW], F32)
    nc.sync.dma_start(x_sb[:], x_d[:])

    q4 = bigs.tile([128, HW], F32)
    k4 = bigs.tile([128, HW], F32)
    wt = bigs.tile([128, MT * 65], F32)   # per m-tile [vT | ones] chunks
    xct = bigs.tile([128, MT * C], F32)   # xcT, m-tile-major
    outc = bigs.tile([C, HW], F32)

    Exp = mybir.ActivationFunctionType.Exp

    # ---- q4 / k4: replicated q,k via stacked-weight 1x1 convs ----
    for which, (wT, dst) in enumerate([(wq4T, q4), (wk4T, k4)]):
        for g in range(3):  # blocks of 3,3,2 n-chunks
            lo = g * 3
            hi = min(lo + 3, NB)
            ps = spool.tile([128, GRP * BLK], F32, tag="s")
            for j in range(hi - lo):
                nc.tensor.matmul(
                    ps[:, j * BLK : (j + 1) * BLK],
                    wT[:],
                    x_sb[:, (lo + j) * BLK : (lo + j + 1) * BLK],
                    start=True,
                    stop=True,
                )
            w = (hi - lo) * BLK
            eng = nc.scalar if which == 0 else nc.vector
            if which == 0:
                eng.copy(dst[:, lo * BLK : lo * BLK + w], ps[:, :w])
            else:
                eng.tensor_copy(dst[:, lo * BLK : lo * BLK + w], ps[:, :w])

    # ---- WT (= [vT | ones]) and xcT, per m-tile, shared stationary x ----
    for g, (base, size) in enumerate(_groups()):
        ps = spool.tile([128, GRP * BLK], F32, tag="s")
        for j in range(size):
            m = base + j
            nc.tensor.matmul(
                ps[:, j * BLK : j * BLK + 129],
                x_sb[:, m * 128 : (m + 1) * 128],
                wvc[:],
                start=True,
                stop=True,
            )
        # strided copies: vT part -> wt, cam part -> xct
        src = ps[:, : size * BLK].rearrange("p (j c) -> p j c", c=BLK)
        wt_dst = wt[:, base * 65 : (base + size) * 65].rearrange(
            "p (j c) -> p j c", c=65
        )
        nc.vector.tensor_copy(wt_dst, src[:, :, 0:65])
        xct_dst = xct[:, base * C : (base + size) * C].rearrange(
            "p (j c) -> p j c", c=C
        )
        nc.scalar.copy(xct_dst, src[:, :, 65:129])
    # ones column (wvc col 64 is zero -> overwrite with 1.0)
    nc.vector.memset(
        wt[:].rearrange("p (t c) -> p t c", c=65)[:, :, 64:65], 1.0
    )

    # ---- CAM: energy_c (symmetric) -> row softmax -> transpose -> out_c ----
    ec = ppool.tile([128, BLK], F32, tag="p")
    for t in range(MT):
        nc.tensor.matmul(
            ec[0:C, 0:C],
            xct[:, t * C : (t + 1) * C],
            xct[:, t * C : (t + 1) * C],
            start=(t == 0),
            stop=(t == MT - 1),
        )
    negmax = campool.tile([C, 1], F32)
    nc.vector.reduce_max(negmax[:], ec[0:C, 0:C], axis=mybir.AxisListType.X, negate=True)
    exp_c = campool.tile([C, C], F32)
    nc.scalar.activation(exp_c[:], ec[0:C, 0:C], Exp, bias=negmax[:])
    sum_c = campool.tile([C, 1], F32)
    nc.vector.reduce_sum(sum_c[:], exp_c[:], axis=mybir.AxisListType.X)
    rec_c = campool.tile([C, 1], F32)
    nc.vector.reciprocal(rec_c[:], sum_c[:])
    attn_c = campool.tile([C, C], F32)
    nc.vector.tensor_scalar_mul(attn_c[:], exp_c[:], rec_c[:])
    tps = ppool.tile([128, BLK], F32, tag="p")
    nc.tensor.transpose(tps[0:C, 0:C], attn_c[:], ident[:])
    attn_cT = campool.tile([C, C], F32)
    nc.vector.tensor_copy(attn_cT[:], tps[0:C, 0:C])

    for g in range(3):
        lo = g * 3
        hi = min(lo + 3, NB)
        ps = spool.tile([128, GRP * BLK], F32, tag="s")
        for j in range(hi - lo):
            nc.tensor.matmul(
                ps[0:C, j * BLK : (j + 1) * BLK],
                attn_cT[:],
                x_sb[:, (lo + j) * BLK : (lo + j + 1) * BLK],
                start=True,
                stop=True,
            )
        w = (hi - lo) * BLK
        nc.scalar.copy(outc[:, lo * BLK : lo * BLK + w], ps[0:C, :w])

    # ---- SAM main loop over 8 n-blocks ----
    for nb in range(NB):
        ncol = slice(nb * BLK, (nb + 1) * BLK)
        vacc = vpool.tile([128, BLK], F32, tag="v")
        for base, size in _groups():
            s_t = spool.tile([128, GRP * BLK], F32, tag="s")
            for j in range(size):
                m = base + j
                nc.tensor.matmul(
                    s_t[:, j * BLK : (j + 1) * BLK],
                    k4[32 * j : 32 * j + 32, m * 128 : (m + 1) * 128],
                    q4[32 * j : 32 * j + 32, ncol],
                    start=True,
                    stop=True,
                    tile_position=(32 * j, 0),
                )
            w = size * BLK
            e_t = epool.tile([128, GRP * BLK], F32, tag="e")
            nc.scalar.activation(e_t[:, :w], s_t[:, :w], Exp, bias=zbias[:])
            for j in range(size):
                m = base + j
                nc.tensor.matmul(
                    vacc[0 : C + 1, :],
                    wt[:, m * 65 : (m + 1) * 65],
                    e_t[:, j * BLK : (j + 1) * BLK],
                    start=(m == 0),
                    stop=(m == MT - 1),
                )
        # ---- per-block epilogue ----
        sam_un = sampool.tile([C, BLK], F32)
        nc.vector.tensor_copy(sam_un[:], vacc[0:C, :])
        rz = sampool.tile([128, BLK], F32, tag="rz")
        nc.vector.reciprocal(rz[C : C + 1, :], vacc[C : C + 1, :])
        bc = ppool.tile([128, BLK], F32, tag="p")
        nc.tensor.matmul(
            bc[0:C, :],
            ones_r[C : C + 1, 0:C],
            rz[C : C + 1, :],
            start=True,
            stop=True,
            tile_position=(C, 0),
        )
        sam_sc = sampool.tile([C, BLK], F32)
        nc.vector.tensor_mul(sam_sc[:], sam_un[:], bc[0:C, :])
        bn = ppool.tile([128, BLK], F32, tag="p")
        nc.tensor.matmul(bn[0:C, :], wbn1T[:], outc[:, ncol], start=True, stop=False)
        nc.tensor.matmul(bn[0:C, :], wbn2T[:], sam_sc[:], start=False, stop=True)
        o_t = outpool.tile([C, BLK], F32)
        nc.vector.tensor_add(o_t[:], x_sb[:, ncol], bn[0:C, :])
        nc.sync.dma_start(out_d[:, ncol], o_t[:])


def build_nc():
    nc = bacc.Bacc(
        "TRN2",
        target_bir_lowering=False,
        debug=False,
        enable_asserts=False,
        num_devices=8,
    )
    io = {}
    io["x"] = nc.dram_tensor("x", [C, HW], F32, kind="ExternalInput").ap()
    io["wq4T"] = nc.dram_tensor("wq4T", [C, 128], F32, kind="ExternalInput").ap()
    io["wk4T"] = nc.dram_tensor("wk4T", [C, 128], F32, kind="ExternalInput").ap()
    io["wvc"] = nc.dram_tensor("wvc", [C, 129], F32, kind="ExternalInput").ap()
    io["wbn1T"] = nc.dram_tensor("wbn1T", [C, C], F32, kind="ExternalInput").ap()
    io["wbn2T"] = nc.dram_tensor("wbn2T", [C, C], F32, kind="ExternalInput").ap()
    io["ident"] = nc.dram_tensor("ident", [C, C], F32, kind="ExternalInput").ap()
    io["ones64"] = nc.dram_tensor("ones64", [1, C], F32, kind="ExternalInput").ap()
    io["out"] = nc.dram_tensor("out", [C, HW], F32, kind="ExternalOutput").ap()

    with tile.TileContext(nc) as tc:
        with ExitStack() as ctx:
            _build_kernel(ctx, tc, io)
    nc.compile()
    return nc


def make_in_maps(x, w_cam, w_q, w_k, w_v, w_bn):
    f = lambda a: np.ascontiguousarray(np.asarray(a, dtype=np.float32))
    base = {
        "wq4T": f(np.concatenate([np.asarray(w_q).T] * 4, axis=1)),
        "wk4T": f(np.concatenate([np.asarray(w_k).T] * 4, axis=1)),
        "wvc": f(
            np.concatenate(
                [np.asarray(w_v).T, np.zeros((C, 1), np.float32), np.asarray(w_cam).T],
                axis=1,
            )
        ),
        "wbn1T": f(np.asarray(w_bn)[:, :C].T),
        "wbn2T": f(np.asarray(w_bn)[:, C:].T),
        "ident": f(np.eye(C)),
        "ones64": f(np.ones((1, C))),
    }
    x = np.asarray(x)
    return [dict(base, x=f(x[b].reshape(C, HW))) for b in range(8)]


_NC_CACHE = None


def kernel(x, w_cam, w_q, w_k, w_v, w_bn):
    global _NC_CACHE
    if _NC_CACHE is None:
        _NC_CACHE = build_nc()
    nc = _NC_CACHE
    in_maps = make_in_maps(x, w_cam, w_q, w_k, w_v, w_bn)
    res = run_bass_kernel_spmd(nc, in_maps, list(range(8)))
    out = np.stack([res.results[b]["out"].reshape(C, 64, 64) for b in range(8)])
    return out.astype(np.float32)
